# revision 1
# baseline (speedup 1.0000x reference)
"""Trainium2 Bass kernel for nn_HINGE_56985626083396 (dense_cnn).

Data-parallel over batch on 8 NeuronCores.

Host precomputes embedding x conv-weight product tables (a weight/embedding
transform, independent of the batch):
  TROLE[r] = [emb_roles[r] @ W1r.T | @ W2fr0.T | @ W2kr.T]        [50000, 1536]
  TVAL[v]  = [emb_values[v] @ W1v.T | @ W2fv0.T | @ W1v.T | @ W2fv1.T
              | @ W2kv.T]                                          [50000, 2560]

Device per core (bl = 4096 batch, 8 chunks of nb=512):
  phase 1: 5 transposed dma_gathers per chunk (fr0, fv0, fv1, kr x4, kv x4)
           from the product tables; window outputs are pure adds (DVE);
           per-(window, filter-chunk) sums (DVE reduce) and sums of squares
           (ACT Square accum); spill pre-BN outputs (bf16) to DRAM with
           4KB-per-partition descriptors.
  AllReduce (8 cores) of the BN sufficient statistics.
  phase 2: reload spill, BN affine per (window, fc) on ACT, min over the
           6 windows + relu on DVE, FC dot on PE -> [bl] f32.

int16 gather indices address the full 50000-row table via a mid-table
base (row 32768) + signed offsets; each index stream is padded with
positive sentinels because the gather ucode trims trailing-negative
indices.
"""

import numpy as np
import ml_dtypes

from concourse import bass, bacc, mybir
import concourse.tile as tile
from concourse.bass_utils import run_bass_kernel_spmd
from concourse.library_config import mlp

CORES = 8
B_FULL = 32768
E = 256
F = 512
NVOC = 50000
BASE = 32768          # mid-table base row for signed int16 indexing
ARITY = 6
BN_EPS = 1e-5
NB = 512              # batch chunk
PAD = 128             # gather index padding (num_idxs must be %128)
SENT = NVOC - 1       # sentinel row (positive as int16 after -BASE)

CR = 1536             # TROLE cols
CV = 2560             # TVAL cols
NF_IDX = NB + PAD             # fr0/fv0/fv1 idx count
NK_IDX = 2 * NB + PAD         # kr/kv idx count (2 windows per gather)

bf16 = mybir.dt.bfloat16
f32 = mybir.dt.float32
i16 = mybir.dt.int16
AF = mybir.ActivationFunctionType
OP = mybir.AluOpType


def build_nc(bl, nb=NB, debug=False):
    nchunk = bl // nb
    # NOTE: num_swdge_queues > 1 silently corrupts concurrent gathers on
    # this ucode; single_packet=True crashes with transpose. Keep defaults.
    nc = bacc.Bacc("TRN2", target_bir_lowering=False, debug=False,
                   num_devices=CORES)

    trole = nc.dram_tensor("trole", [NVOC, CR], bf16, kind="ExternalInput")
    tval = nc.dram_tensor("tval", [NVOC, CV], bf16, kind="ExternalInput")
    fcw_d = nc.dram_tensor("fcw", [128, 4], bf16, kind="ExternalInput")
    fcb_d = nc.dram_tensor("fcb", [1, 1], f32, kind="ExternalInput")
    gam_d = nc.dram_tensor("gamma_x", [128, 24], f32, kind="ExternalInput")
    bet_d = nc.dram_tensor("beta_x", [128, 24], f32, kind="ExternalInput")
    # packed per-chunk wrapped indices: [fr0 | fv0 | fv1] and [kr | kv]
    idxf_d = nc.dram_tensor("idx_f", [nchunk * 128, 3 * (NF_IDX // 16)], i16,
                            kind="ExternalInput")
    idxk_d = nc.dram_tensor("idx_k", [nchunk * 128, 4 * (NK_IDX // 16)], i16,
                            kind="ExternalInput")
    out_d = nc.dram_tensor("out", [1, bl], f32, kind="ExternalOutput")
    # spill: per (window, chunk) a [128, 4, nb] block, 4KB/partition contig
    spill = nc.dram_tensor("spill", [6, nchunk, 128, 4, nb], bf16,
                           kind="ExternalOutput" if debug else "Internal")
    if debug:
        dbg_stats = nc.dram_tensor("dbg_stats", [128, 48], f32,
                                   kind="ExternalOutput")
        dbg_ar = nc.dram_tensor("dbg_ar", [128, 48], f32,
                                kind="ExternalOutput")
        dbg_gfr = nc.dram_tensor("dbg_gfr", [128, 8, NF_IDX], bf16,
                                 kind="ExternalOutput")
        dbg_gv0 = nc.dram_tensor("dbg_gv0", [128, 8, NF_IDX], bf16,
                                 kind="ExternalOutput")
        dbg_gkr = nc.dram_tensor("dbg_gkr", [128, 4, NK_IDX], bf16,
                                 kind="ExternalOutput")

    nc.gpsimd.load_library(mlp)
    NFW = NF_IDX // 16
    NKW = NK_IDX // 16

    with tile.TileContext(nc) as tc:
        with tc.tile_pool(name="wp", bufs=1) as wp, \
             tc.tile_pool(name="statp", bufs=1) as statp:
            fcw_sb = wp.tile([128, 4], bf16)
            nc.sync.dma_start(out=fcw_sb[:], in_=fcw_d[:])
            fcb_sb = wp.tile([1, 1], f32)
            nc.sync.dma_start(out=fcb_sb[:], in_=fcb_d[:])
            gam_sb = wp.tile([128, 24], f32)
            nc.sync.dma_start(out=gam_sb[:], in_=gam_d[:])
            bet_sb = wp.tile([128, 24], f32)
            nc.sync.dma_start(out=bet_sb[:], in_=bet_d[:])

            # per-chunk stats: col = ch*24 + (w*4 + j)
            sumc = statp.tile([128, nchunk * 24], f32)
            s2c = statp.tile([128, nchunk * 24], f32)

            # ---------------- phase 1 ----------------
            with tc.tile_pool(name="idxp", bufs=3) as idxp, \
                 tc.tile_pool(name="xp", bufs=2) as xp, \
                 tc.tile_pool(name="scrp", bufs=2) as scrp:
                for ch in range(nchunk):
                    ixf = idxp.tile([128, 3 * NFW], i16, tag="ixf")
                    nc.sync.dma_start(out=ixf[:],
                                      in_=idxf_d[ch * 128:(ch + 1) * 128, :])
                    ixk = idxp.tile([128, 4 * NKW], i16, tag="ixk")
                    nc.sync.dma_start(out=ixk[:],
                                      in_=idxk_d[ch * 128:(ch + 1) * 128, :])

                    gfr = xp.tile([128, 8, NF_IDX], bf16, tag="gfr")
                    nc.gpsimd.dma_gather(
                        gfr[:], trole[BASE:, 0:1024], ixf[:, 0:NFW],
                        NF_IDX, NF_IDX, 1024, elem_step=CR,
                        transpose=True, single_packet=False, queue_num=0)
                    gv0 = xp.tile([128, 8, NF_IDX], bf16, tag="gv0")
                    nc.gpsimd.dma_gather(
                        gv0[:], tval[BASE:, 0:1024], ixf[:, NFW:2 * NFW],
                        NF_IDX, NF_IDX, 1024, elem_step=CV,
                        transpose=True, single_packet=False, queue_num=0)
                    gv1 = xp.tile([128, 8, NF_IDX], bf16, tag="gv1")
                    nc.gpsimd.dma_gather(
                        gv1[:], tval[BASE:, 1024:2048], ixf[:, 2 * NFW:3 * NFW],
                        NF_IDX, NF_IDX, 1024, elem_step=CV,
                        transpose=True, single_packet=False, queue_num=0)
                    gkrs, gkvs = [], []
                    for h in range(2):
                        gkr = xp.tile([128, 4, NK_IDX], bf16, tag=f"gkr{h}")
                        nc.gpsimd.dma_gather(
                            gkr[:], trole[BASE:, 1024:1536],
                            ixk[:, h * NKW:(h + 1) * NKW],
                            NK_IDX, NK_IDX, 512, elem_step=CR,
                            transpose=True, single_packet=False,
                            queue_num=0)
                        gkrs.append(gkr)
                        gkv = xp.tile([128, 4, NK_IDX], bf16, tag=f"gkv{h}")
                        nc.gpsimd.dma_gather(
                            gkv[:], tval[BASE:, 2048:2560],
                            ixk[:, (2 + h) * NKW:(3 + h) * NKW],
                            NK_IDX, NK_IDX, 512, elem_step=CV,
                            transpose=True, single_packet=False,
                            queue_num=0)
                        gkvs.append(gkv)

                    if debug and ch == 0:
                        nc.sync.dma_start(out=dbg_gfr[:], in_=gfr[:])
                        nc.sync.dma_start(out=dbg_gv0[:], in_=gv0[:])
                        nc.sync.dma_start(out=dbg_gkr[:], in_=gkr[:])

                    # window outputs computed in place inside the gather
                    # tiles (frees SBUF for full double buffering):
                    #   y0 <- gv0[0:4], y1 <- gv1[0:4], b2 <- gfr[4:8],
                    #   y(2+k) <- gkr[:, k*nb:(k+1)*nb]
                    nc.vector.tensor_tensor(out=gv0[:, 0:4, 0:nb],
                                            in0=gv0[:, 0:4, 0:nb],
                                            in1=gfr[:, 0:4, 0:nb], op=OP.add)
                    nc.vector.tensor_tensor(out=gv1[:, 0:4, 0:nb],
                                            in0=gv1[:, 0:4, 0:nb],
                                            in1=gfr[:, 0:4, 0:nb], op=OP.add)
                    nc.vector.tensor_tensor(out=gfr[:, 4:8, 0:nb],
                                            in0=gv0[:, 4:8, 0:nb],
                                            in1=gfr[:, 4:8, 0:nb], op=OP.add)
                    nc.vector.tensor_tensor(out=gfr[:, 4:8, 0:nb],
                                            in0=gfr[:, 4:8, 0:nb],
                                            in1=gv1[:, 4:8, 0:nb], op=OP.add)
                    ys = [gv0[:, 0:4, 0:nb], gv1[:, 0:4, 0:nb]]
                    for k in range(4):
                        h, r = divmod(k, 2)
                        ksl = slice(r * nb, (r + 1) * nb)
                        gkr, gkv = gkrs[h], gkvs[h]
                        nc.vector.tensor_tensor(
                            out=gkr[:, :, ksl], in0=gfr[:, 4:8, 0:nb],
                            in1=gkr[:, :, ksl], op=OP.add)
                        nc.vector.tensor_tensor(
                            out=gkr[:, :, ksl], in0=gkr[:, :, ksl],
                            in1=gkv[:, :, ksl], op=OP.add)
                        ys.append(gkr[:, :, ksl])

                    for w, y in enumerate(ys):
                        # batch sums for BN mean
                        nc.vector.tensor_reduce(
                            out=sumc[:, ch * 24 + w * 4: ch * 24 + w * 4 + 4],
                            in_=y, axis=mybir.AxisListType.X, op=OP.add)
                        # sums of squares for BN var
                        for j in range(4):
                            sqs = scrp.tile([128, nb], bf16, tag="sqs")
                            col = ch * 24 + w * 4 + j
                            nc.scalar.activation(
                                out=sqs[:], in_=y[:, j, :], func=AF.Square,
                                accum_out=s2c[:, col:col + 1])
                        nc.sync.dma_start(out=spill[w, ch], in_=y)

            # ---------------- stats + allreduce ----------------
            with tc.tile_pool(name="fsp", bufs=1) as fsp, \
                 tc.tile_pool(name="dramp", bufs=1, space="DRAM") as dramp:
                stats = fsp.tile([128, 48], f32)
                nc.vector.tensor_reduce(
                    out=stats[:, 0:24],
                    in_=sumc[:].rearrange("p (c a) -> p a c", a=24),
                    axis=mybir.AxisListType.X, op=OP.add)
                nc.vector.tensor_reduce(
                    out=stats[:, 24:48],
                    in_=s2c[:].rearrange("p (c a) -> p a c", a=24),
                    axis=mybir.AxisListType.X, op=OP.add)

                cc_in = dramp.tile([128, 48], f32)
                cc_out = dramp.tile([128, 48], f32)
                nc.sync.dma_start(out=cc_in[:], in_=stats[:])
                ar = fsp.tile([128, 48], f32)
                nc.gpsimd.collective_compute(
                    "AllReduce", OP.add,
                    replica_groups=[list(range(CORES))],
                    ins=[cc_in[:].opt()], outs=[cc_out[:].opt()])
                nc.sync.dma_start(out=ar[:], in_=cc_out[:])
                if debug:
                    nc.sync.dma_start(out=dbg_stats[:], in_=stats[:])
                    nc.sync.dma_start(out=dbg_ar[:], in_=ar[:])

                inv_n = 1.0 / (bl * CORES)
                mean = fsp.tile([128, 24], f32)
                nc.scalar.mul(out=mean[:], in_=ar[:, 0:24], mul=inv_n)
                ex2 = fsp.tile([128, 24], f32)
                nc.scalar.mul(out=ex2[:], in_=ar[:, 24:48], mul=inv_n)
                var = fsp.tile([128, 24], f32)
                nc.vector.tensor_tensor(out=var[:], in0=mean[:], in1=mean[:],
                                        op=OP.mult)
                nc.vector.tensor_tensor(out=var[:], in0=ex2[:], in1=var[:],
                                        op=OP.subtract)
                nc.vector.tensor_scalar_add(out=var[:], in0=var[:],
                                            scalar1=BN_EPS)
                std = fsp.tile([128, 24], f32)
                nc.scalar.activation(out=std[:], in_=var[:], func=AF.Sqrt,
                                     bias=0.0, scale=1.0)
                rstd = fsp.tile([128, 24], f32)
                nc.vector.reciprocal(out=rstd[:], in_=std[:])
                avec = fsp.tile([128, 24], f32)
                nc.vector.tensor_tensor(out=avec[:], in0=rstd[:], in1=gam_sb[:],
                                        op=OP.mult)
                cvec = fsp.tile([128, 24], f32)
                nc.vector.tensor_tensor(out=cvec[:], in0=mean[:], in1=avec[:],
                                        op=OP.mult)
                nc.vector.tensor_tensor(out=cvec[:], in0=bet_sb[:], in1=cvec[:],
                                        op=OP.subtract)

                # ---------------- phase 2 ----------------
                # affine split across ACT (w 0-2, relu fused) and DVE
                # (w 3-5, no relu); mins on 3D [128, 4, nb] tiles; the
                # final relu after the min fixes the DVE windows (exact:
                # relu(min(relu(a), b)) == relu(min(a, b))).
                with tc.tile_pool(name="ldp", bufs=2) as ldp, \
                     tc.tile_pool(name="zp", bufs=2) as zp, \
                     tc.tile_pool(name="mp", bufs=2) as mp, \
                     tc.tile_pool(name="orow", bufs=1) as orow, \
                     tc.tile_pool(name="ps2", bufs=2, space="PSUM") as ps2:
                    outrow = orow.tile([1, bl], f32)
                    for ch in range(nchunk):
                        zs = []
                        for w in range(6):
                            ld = ldp.tile([128, 4, nb], bf16, tag=f"ld{w}")
                            nc.sync.dma_start(out=ld[:], in_=spill[w, ch])
                            z = zp.tile([128, 4, nb], bf16, tag=f"z{w}")
                            if w < 3:
                                for j in range(4):
                                    col = w * 4 + j
                                    nc.scalar.activation(
                                        out=z[:, j, :], in_=ld[:, j, :],
                                        func=AF.Relu,
                                        scale=avec[:, col:col + 1],
                                        bias=cvec[:, col:col + 1])
                            else:
                                for j in range(4):
                                    col = w * 4 + j
                                    nc.vector.tensor_scalar(
                                        out=z[:, j, :], in0=ld[:, j, :],
                                        scalar1=avec[:, col:col + 1],
                                        scalar2=cvec[:, col:col + 1],
                                        op0=OP.mult, op1=OP.add)
                            zs.append(z)
                        acc = mp.tile([128, 4, nb], bf16, tag="acc")
                        nc.vector.tensor_tensor(out=acc[:], in0=zs[0][:],
                                                in1=zs[1][:], op=OP.min)
                        for w in range(2, 6):
                            nc.vector.tensor_tensor(out=acc[:], in0=acc[:],
                                                    in1=zs[w][:], op=OP.min)
                        relu = mp.tile([128, 4, nb], bf16, tag="relu")
                        nc.vector.tensor_scalar_max(out=relu[:], in0=acc[:],
                                                    scalar1=0.0)
                        pfc = ps2.tile([1, nb], f32, tag="pfc")
                        for j in range(4):
                            nc.tensor.matmul(out=pfc[:],
                                             lhsT=fcw_sb[:, j:j + 1],
                                             rhs=relu[:, j, :],
                                             start=(j == 0), stop=(j == 3))
                        nc.vector.tensor_scalar_add(
                            out=outrow[:, ch * nb:(ch + 1) * nb],
                            in0=pfc[:], scalar1=fcb_sb[:1, :1])
                    nc.sync.dma_start(out=out_d[:], in_=outrow[:])

    nc.compile()
    return nc


# ---------------- host side ----------------

def _wrap16(a):
    """int16 index array [n] -> [128, n/16] wrapped layout."""
    w = a.reshape(-1, 16).T
    return np.tile(w, (8, 1))


def _enc(idx, n_pad):
    """int64 vocab indices -> padded signed-offset int16."""
    out = np.full(n_pad, SENT, np.int64)
    out[:len(idx)] = idx
    return (out - BASE).astype(np.int16)


def _product_tables(emb_roles, emb_values, conv1_w, conv2_w):
    er = np.asarray(emb_roles, np.float32)
    ev = np.asarray(emb_values, np.float32)
    c1 = np.asarray(conv1_w, np.float32)
    c2 = np.asarray(conv2_w, np.float32)
    w1v, w1r = c1[:, 0, :], c1[:, 1, :]
    w2fv0, w2fr0, w2fv1, w2kr, w2kv = (c2[:, i, :] for i in range(5))
    p_w1r = er @ w1r.T
    p_w2fr0 = er @ w2fr0.T
    p_w2kr = er @ w2kr.T
    p_w1v = ev @ w1v.T
    p_w2fv0 = ev @ w2fv0.T
    p_w2fv1 = ev @ w2fv1.T
    p_w2kv = ev @ w2kv.T
    trole = np.concatenate([p_w1r, p_w2fr0, p_w2kr], axis=1)
    tval = np.concatenate([p_w1v, p_w2fv0, p_w1v, p_w2fv1, p_w2kv], axis=1)
    return trole.astype(ml_dtypes.bfloat16), tval.astype(ml_dtypes.bfloat16)


def _expand_bn(v1, v2):
    """bn1/bn2 [F] -> [128, 24] per (window, fc)."""
    out = np.empty((128, 6, 4), np.float32)
    for w in range(6):
        src = v1 if w < 2 else v2
        out[:, w, :] = np.asarray(src, np.float32).reshape(4, 128).T
    return out.reshape(128, 24)


_CACHE = {}


def _get_nc(bl):
    if bl not in _CACHE:
        _CACHE[bl] = build_nc(bl)
    return _CACHE[bl]


def make_in_maps(x_batch, emb_roles, emb_values, conv1_w, conv2_w,
                 bn1_gamma, bn1_beta, bn2_gamma, bn2_beta, fc_w, fc_b,
                 bl, nb=NB):
    nchunk = bl // nb
    trole, tval = _product_tables(emb_roles, emb_values, conv1_w, conv2_w)
    shared = {
        "trole": trole,
        "tval": tval,
        "fcw": np.asarray(fc_w, np.float32).reshape(4, 128).T
                 .astype(ml_dtypes.bfloat16),
        "fcb": np.asarray(fc_b, np.float32).reshape(1, 1),
        "gamma_x": _expand_bn(bn1_gamma, bn2_gamma),
        "beta_x": _expand_bn(bn1_beta, bn2_beta),
    }
    xb = np.asarray(x_batch).astype(np.int64)
    in_maps = []
    for c in range(CORES):
        xs = xb[c * bl:(c + 1) * bl]
        roles = xs[:, 0::2]
        values = xs[:, 1::2]
        frows, krows = [], []
        for ch in range(nchunk):
            sl = slice(ch * nb, (ch + 1) * nb)
            fr0 = _enc(roles[sl, 0], NF_IDX)
            fv0 = _enc(values[sl, 0], NF_IDX)
            fv1 = _enc(values[sl, 1], NF_IDX)
            kr01 = _enc(roles[sl, 2:4].T.reshape(-1), NK_IDX)
            kr23 = _enc(roles[sl, 4:6].T.reshape(-1), NK_IDX)
            kv01 = _enc(values[sl, 2:4].T.reshape(-1), NK_IDX)
            kv23 = _enc(values[sl, 4:6].T.reshape(-1), NK_IDX)
            frows.append(np.concatenate(
                [_wrap16(fr0), _wrap16(fv0), _wrap16(fv1)], axis=1))
            krows.append(np.concatenate(
                [_wrap16(kr01), _wrap16(kr23),
                 _wrap16(kv01), _wrap16(kv23)], axis=1))
        m = dict(shared)
        m.update({"idx_f": np.concatenate(frows, axis=0),
                  "idx_k": np.concatenate(krows, axis=0)})
        in_maps.append(m)
    return in_maps


def kernel(x_batch, arity, emb_roles, emb_values,
           conv1_w, conv1_b, bn1_gamma, bn1_beta,
           conv2_w, conv2_b, bn2_gamma, bn2_beta, fc_w, fc_b):
    # conv biases cancel exactly under training-mode batchnorm.
    bl = np.asarray(x_batch).shape[0] // CORES
    nc = _get_nc(bl)
    in_maps = make_in_maps(x_batch, emb_roles, emb_values, conv1_w, conv2_w,
                           bn1_gamma, bn1_beta, bn2_gamma, bn2_beta,
                           fc_w, fc_b, bl)
    res = run_bass_kernel_spmd(nc, in_maps, core_ids=list(range(CORES)))
    out = np.concatenate([res.results[c]["out"].reshape(bl, 1)
                          for c in range(CORES)], 0)
    return out.astype(np.float32)

